# revision 2
# baseline (speedup 1.0000x reference)
"""CRF forward-algorithm (log partition) kernel for 8 Trainium2 NeuronCores.

Strategy: segment-spliced exp-space scan.

The reference recurrence  fv' = logsumexp_prev(fv + T) + feat  is, in exp
space, a linear matvec chain  v' = diag(e_t) @ M @ v  with M = exp(T) fixed.
We split the T=16384 steps into S=1024 segments of L=16 and run all segments
in parallel from a guess vector, batched 129 columns per core so the PE array
runs dense [128x128] x [128x129] matmuls (full utilization) instead of
matvecs.  Products of positive matrices contract exponentially toward rank-1
(Perron-Frobenius), so the true correction at each segment junction is a pure
scalar kappa, measured exactly by re-running only the first D=8 steps of each
segment from the previous segment's endpoint (phase 2, also fully parallel —
logsumexp commutes with additive constants).  alpha = lse(final) + sum(kappa).

Per-step rescaling is folded into the emissions as a constant e^-8 (zero
cost); all bookkeeping scales are recovered analytically at the end.

Each core is fully independent (no collectives): core c owns segments
[c*128, c*128+128] (129 columns, one redundant boundary column so junction
sources are always core-local).  The host does the tiny O(S*N) final
assembly (kappa extraction + terminal logsumexp) in fp64.
"""

import numpy as np
import ml_dtypes

import concourse.bass as bass
import concourse.bacc as bacc
import concourse.mybir as mybir
import concourse.tile as tile

BF16_NP = ml_dtypes.bfloat16
BF16 = mybir.dt.bfloat16
F32 = mybir.dt.float32

SEQ_LEN = 16384
N_TAGS = 1024
START_IDX = 1022
STOP_IDX = 1023
NB = 8                 # 1024 tags = 8 blocks of 128 partitions
L = 16                 # segment length (steps)
D = 8                  # junction fixup depth (steps)
S = SEQ_LEN // L       # 1024 segments
NCORES = 8
BPC = S // NCORES      # 128 segments owned per core
NCOLS = BPC + 1        # 129 phase-1 columns (1 redundant boundary col)
CSCALE = 8.0           # constant per-step rescale folded into emissions

_CACHE = {}


def _build_program():
    nc = bacc.Bacc("TRN2", target_bir_lowering=False, debug=False)
    mt = nc.dram_tensor("mt", [N_TAGS, N_TAGS], BF16, kind="ExternalInput")
    vinit = nc.dram_tensor("vinit", [N_TAGS, NCOLS], BF16, kind="ExternalInput")
    e1 = nc.dram_tensor("e1", [L, 128, NB * NCOLS], BF16, kind="ExternalInput")
    e2 = nc.dram_tensor("e2", [D, 128, NB * BPC], BF16, kind="ExternalInput")
    snap = nc.dram_tensor("snap", [NB, 128, NCOLS], BF16, kind="ExternalOutput")
    yend = nc.dram_tensor("yend", [NB, 128, NCOLS], BF16, kind="ExternalOutput")
    zout = nc.dram_tensor("zout", [NB, 128, BPC], BF16, kind="ExternalOutput")

    with tile.TileContext(nc) as tc:
        with (
            tc.tile_pool(name="mpool", bufs=1) as mpool,
            tc.tile_pool(name="vpool", bufs=2) as vpool,
            tc.tile_pool(name="epool", bufs=3) as epool,
            tc.tile_pool(name="pspool", bufs=1, space="PSUM") as pspool,
        ):
            # Stationary operand: mt[prev, next]; section kb holds rows
            # [kb*128, kb*128+128) across all next-tags.
            mt_sb = mpool.tile([128, NB * N_TAGS], BF16)
            for kb in range(NB):
                nc.sync.dma_start(
                    mt_sb[:, kb * N_TAGS:(kb + 1) * N_TAGS],
                    mt[kb * 128:(kb + 1) * 128, :],
                )

            v_tiles = []
            for kb in range(NB):
                vt = vpool.tile([128, NCOLS], BF16, tag=f"v{kb}")
                nc.sync.dma_start(vt[:], vinit[kb * 128:(kb + 1) * 128, :])
                v_tiles.append(vt)

            def step(v_aps, e_row, ncols, out_dram=None):
                et = epool.tile([128, NB * ncols], BF16, tag="e")
                nc.sync.dma_start(et[:], e_row)
                new_tiles = []
                for mb in range(NB):
                    ps = pspool.tile([128, ncols], F32, tag=f"ps{mb}")
                    for kb in range(NB):
                        sec = kb * N_TAGS + mb * 128
                        nc.tensor.matmul(
                            ps[:],
                            mt_sb[:, sec:sec + 128],
                            v_aps[kb],
                            start=(kb == 0),
                            stop=(kb == NB - 1),
                        )
                    nv = vpool.tile([128, ncols], BF16, tag=f"v{mb}")
                    nc.vector.tensor_mul(
                        nv[:], ps[:], et[:, mb * ncols:(mb + 1) * ncols]
                    )
                    if out_dram is not None:
                        nc.sync.dma_start(out_dram[mb], nv[:])
                    new_tiles.append(nv)
                return new_tiles

            for s in range(L):
                out_d = snap if s + 1 == D else (yend if s + 1 == L else None)
                v_tiles = step([vt[:] for vt in v_tiles], e1[s], NCOLS, out_d)

            v_aps = [vt[:, 0:BPC] for vt in v_tiles]
            for s in range(D):
                out_d = zout if s + 1 == D else None
                new = step(v_aps, e2[s], BPC, out_d)
                v_aps = [vt[:] for vt in new]

    nc.compile()
    return nc


def _prepare_core_inputs(E, Mt_bf, vinit_base):
    """Per-core input dicts. E: [T, N] bf16 emissions exp(decoded - CSCALE)."""
    in_maps = []
    steps1 = np.arange(L)
    steps2 = np.arange(D)
    for c in range(NCORES):
        segs1 = np.minimum(c * BPC + np.arange(NCOLS), S - 1)
        segs2 = np.minimum(c * BPC + 1 + np.arange(BPC), S - 1)
        t1 = segs1 * L  # [NCOLS]
        t2 = segs2 * L  # [BPC]
        # a1[s, col, tag] -> e1[s, p, mb*NCOLS + col]
        a1 = E[t1[None, :] + steps1[:, None]]          # [L, NCOLS, N]
        a1 = a1.reshape(L, NCOLS, NB, 128)
        e1 = np.ascontiguousarray(a1.transpose(0, 3, 2, 1)).reshape(L, 128, NB * NCOLS)
        a2 = E[t2[None, :] + steps2[:, None]]          # [D, BPC, N]
        a2 = a2.reshape(D, BPC, NB, 128)
        e2 = np.ascontiguousarray(a2.transpose(0, 3, 2, 1)).reshape(D, 128, NB * BPC)
        vin = vinit_base.copy()
        if c == 0:
            vin[:, 0] = BF16_NP(0.0)
            vin[START_IDX, 0] = BF16_NP(1.0)
        in_maps.append({"mt": Mt_bf, "vinit": vin, "e1": e1, "e2": e2})
    return in_maps


def _assemble(transitions, results):
    """Host-side kappa extraction + terminal logsumexp (tiny, fp64)."""
    kappa_sum = 0.0
    max_spread = 0.0
    for c in range(NCORES):
        snap = results[c]["snap"].astype(np.float64)  # [NB, 128, NCOLS]
        zout = results[c]["zout"].astype(np.float64)  # [NB, 128, BPC]
        # col j of zout: junction for segment c*BPC+j+1; compare with snap col j+1
        nj = BPC if c < NCORES - 1 else BPC - 1  # core 7's last junction is dummy
        z = zout.reshape(N_TAGS, BPC)[:, :nj]
        sn = snap.reshape(N_TAGS, NCOLS)[:, 1:nj + 1]
        valid = (z > 0) & (sn > 0)
        with np.errstate(divide="ignore", invalid="ignore"):
            dlt = np.where(valid, np.log(z) - np.log(sn), np.nan)
        kap = np.nanmedian(dlt, axis=0)
        spread = np.nanmax(dlt, axis=0) - np.nanmin(dlt, axis=0)
        max_spread = max(max_spread, float(spread.max()))
        kappa_sum += float(kap.sum())

    y_last = results[NCORES - 1]["yend"].astype(np.float64).reshape(N_TAGS, NCOLS)[:, BPC - 1]
    with np.errstate(divide="ignore"):
        logx = np.log(y_last) + kappa_sum + CSCALE * SEQ_LEN
    term = logx + transitions[STOP_IDX].astype(np.float64)
    term = term[np.isfinite(term)]
    mx = term.max()
    alpha = mx + np.log(np.exp(term - mx).sum())
    return alpha, max_spread


def kernel(decoded, transitions, raw_outputs=None, outputs=None, _backend="hw"):
    decoded = np.asarray(decoded, dtype=np.float32)
    transitions = np.asarray(transitions, dtype=np.float32)

    Mt_bf = np.exp(transitions.astype(np.float64)).T.astype(BF16_NP)  # [prev, next]
    Mt_bf = np.ascontiguousarray(Mt_bf)
    E = np.exp(decoded - np.float32(CSCALE)).astype(BF16_NP)          # [T, N]
    vinit_base = np.full((N_TAGS, NCOLS), 1.0 / N_TAGS, dtype=BF16_NP)

    in_maps = _prepare_core_inputs(E, Mt_bf, vinit_base)
    _CACHE["in_maps"] = in_maps

    if "nc" not in _CACHE:
        _CACHE["nc"] = _build_program()
    nc = _CACHE["nc"]

    if _backend == "sim":
        from concourse.bass_interp import CoreSim
        results = []
        for c in range(NCORES):
            sim = CoreSim(nc, trace=False)
            for k, v in in_maps[c].items():
                sim.tensor(k)[:] = v
            sim.simulate()
            results.append({k: np.array(sim.tensor(k)) for k in ("snap", "yend", "zout")})
    else:
        from concourse.bass_utils import run_bass_kernel_spmd
        res = run_bass_kernel_spmd(nc, in_maps, list(range(NCORES)))
        results = res.results

    alpha, max_spread = _assemble(transitions, results)
    if max_spread > 0.2:
        import sys
        print(f"kernel: WARNING junction spread {max_spread:.3e}", file=sys.stderr)
    return np.float32(alpha)



# revision 5
# speedup vs baseline: 1.3920x; 1.3920x over previous
"""CRF forward-algorithm (log partition) kernel for 8 Trainium2 NeuronCores.

Strategy: segment-spliced exp-space scan (v2).

The reference recurrence  fv' = logsumexp_prev(fv + T) + feat  is, in exp
space, a linear matvec chain  v' = (M @ v) .* e_t  with M = exp(T) fixed.
We split the T=16384 steps into S=1024 segments of L=16 and run all segments
in parallel from a guess vector, batched 129 columns per core so the PE array
runs dense [128x128] x [128x129] matmuls.  Products of positive matrices
contract toward rank-1 (contraction factor ~0.04/step here), so the true
correction at each segment junction is a pure scalar kappa, measured by
re-running only the first D steps of each segment from the previous
segment's endpoint.  alpha = lse(final) + sum(kappa).

v2 refinements over the baseline:
  - Step 0 of every segment is computed on the host: all segments start
    from the uniform guess, so state-after-step-0 = rowsum(M)/N .* e_0,
    an elementwise product.  Device phase 1 runs steps 1..15 only.
  - Fixup depth D=2 (was 8; junction spread at D=8 was ~1.5e-2, far
    below what the tolerance needs).  The LAST fixup step only computes
    tag-block 0 (8 matmuls instead of 64): the junction kappa is a
    median over tags and 128 tags is plenty.
  - PE warm-up matmuls run during the initial 2MB weight DMA so the HAM
    clock-gate is at 2.4GHz when the real stream starts.
  - The first device step is kb-major so matmuls start as soon as the
    first weight section lands instead of after the full 2MB.
  - Input DMAs ride the sync-engine HWDGE queue, outputs + early tiles
    the scalar-engine queue.

Per-step rescaling is folded into the emissions as a constant e^-8; all
bookkeeping scales are recovered analytically at the end.  Each core is
fully independent (no collectives): core c owns segments [c*128, c*128+128]
(129 columns, one redundant boundary column so junction sources are always
core-local).  The host does the tiny O(S*N) final assembly in fp64.
"""

import numpy as np
import ml_dtypes

import concourse.bass as bass
import concourse.bacc as bacc
import concourse.mybir as mybir
import concourse.tile as tile

BF16_NP = ml_dtypes.bfloat16
BF16 = mybir.dt.bfloat16
F32 = mybir.dt.float32

SEQ_LEN = 16384
N_TAGS = 1024
START_IDX = 1022
STOP_IDX = 1023
NB = 8                 # 1024 tags = 8 blocks of 128 partitions
L = 16                 # segment length (steps)
D = 2                  # junction fixup depth (steps, >= 1)
S = SEQ_LEN // L       # 1024 segments
NCORES = 8
BPC = S // NCORES      # 128 segments owned per core
NCOLS = BPC + 1        # 129 phase-1 columns (1 redundant boundary col)
CSCALE = 8.0           # constant per-step rescale folded into emissions
NWARM = 104            # PE warm-up matmuls issued during the initial DMA

_CACHE = {}


def _build_program():
    nc = bacc.Bacc("TRN2", target_bir_lowering=False, debug=False)
    mt = nc.dram_tensor("mt", [N_TAGS, N_TAGS], BF16, kind="ExternalInput")
    v1 = nc.dram_tensor("v1", [N_TAGS, NCOLS], BF16, kind="ExternalInput")
    e1 = nc.dram_tensor("e1", [L - 1, 128, NB * NCOLS], BF16, kind="ExternalInput")
    if D >= 2:
        e2f = nc.dram_tensor("e2f", [D - 1, 128, NB * BPC], BF16, kind="ExternalInput")
        snap = nc.dram_tensor("snap", [128, NCOLS], BF16, kind="ExternalOutput")
    e2l = nc.dram_tensor("e2l", [128, BPC], BF16, kind="ExternalInput")
    yend = nc.dram_tensor("yend", [NB, 128, NCOLS], BF16, kind="ExternalOutput")
    zout = nc.dram_tensor("zout", [128, BPC], BF16, kind="ExternalOutput")

    with tile.TileContext(nc) as tc:
        with (
            tc.tile_pool(name="mpool", bufs=1) as mpool,
            tc.tile_pool(name="vpool", bufs=2) as vpool,
            tc.tile_pool(name="epool", bufs=3) as epool,
            tc.tile_pool(name="pspool", bufs=1, space="PSUM") as pspool,
        ):
            # --- PE warm-up: keep the HAM clock-gate open during the load.
            warm = mpool.tile([128, 128], BF16, tag="warm")
            nc.vector.memset(warm[:], 0.0)
            wps = pspool.tile([128, NCOLS], F32, tag="ps0")
            for _ in range(NWARM):
                nc.tensor.matmul(wps[:, 0:128], warm[:], warm[:],
                                 start=True, stop=True)

            # --- input DMAs.  mt section kb interleaved with v-tile kb so
            # the kb-major first step can start on section 0 immediately.
            mt_sb = mpool.tile([128, NB * N_TAGS], BF16)
            v_tiles = [vpool.tile([128, NCOLS], BF16, tag=f"v{kb}", name=f"v{kb}")
                       for kb in range(NB)]
            for kb in range(NB):
                nc.sync.dma_start(
                    mt_sb[:, kb * N_TAGS:(kb + 1) * N_TAGS],
                    mt[kb * 128:(kb + 1) * 128, :],
                )
                nc.scalar.dma_start(v_tiles[kb][:], v1[kb * 128:(kb + 1) * 128, :])

            # --- first device step (global step 1), kb-major.
            et0 = epool.tile([128, NB * NCOLS], BF16, tag="e")
            nc.scalar.dma_start(et0[:], e1[0])
            ps_list = [pspool.tile([128, NCOLS], F32, tag=f"ps{mb}", name=f"ps{mb}")
                       for mb in range(NB)]
            for kb in range(NB):
                for mb in range(NB):
                    sec = kb * N_TAGS + mb * 128
                    nc.tensor.matmul(
                        ps_list[mb][:], mt_sb[:, sec:sec + 128], v_tiles[kb][:],
                        start=(kb == 0), stop=(kb == NB - 1),
                    )
            new_tiles = []
            for mb in range(NB):
                nv = vpool.tile([128, NCOLS], BF16, tag=f"v{mb}")
                nc.vector.tensor_mul(
                    nv[:], ps_list[mb][:], et0[:, mb * NCOLS:(mb + 1) * NCOLS])
                if D == 2 and mb == 0:
                    nc.scalar.dma_start(snap[:, :], nv[:])
                new_tiles.append(nv)
            v_tiles = new_tiles

            def full_step(v_aps, e_row, ncols, out_dram=None, e_eng=None,
                          snap_out=None):
                et = epool.tile([128, NB * ncols], BF16, tag="e")
                (e_eng or nc.sync).dma_start(et[:], e_row)
                new = []
                for mb in range(NB):
                    ps = pspool.tile([128, NCOLS], F32, tag=f"ps{mb}")
                    for kb in range(NB):
                        sec = kb * N_TAGS + mb * 128
                        nc.tensor.matmul(
                            ps[:, 0:ncols], mt_sb[:, sec:sec + 128], v_aps[kb],
                            start=(kb == 0), stop=(kb == NB - 1),
                        )
                    nv = vpool.tile([128, NCOLS], BF16, tag=f"v{mb}")
                    nc.vector.tensor_mul(
                        nv[:, 0:ncols], ps[:, 0:ncols],
                        et[:, mb * ncols:(mb + 1) * ncols])
                    if out_dram is not None:
                        nc.scalar.dma_start(out_dram[mb], nv[:, 0:ncols])
                    if snap_out is not None and mb == 0:
                        nc.scalar.dma_start(snap_out, nv[:, 0:ncols])
                    new.append(nv)
                return new

            # --- phase-1 device steps 2..15 (e1 rows 1..14).
            for r in range(1, L - 1):
                snap_out = snap[:, :] if (D >= 2 and r == D - 2) else None
                out_d = [yend[mb] for mb in range(NB)] if r == L - 2 else None
                v_tiles = full_step(
                    [vt[:] for vt in v_tiles], e1[r], NCOLS,
                    out_dram=out_d,
                    e_eng=(nc.scalar if r % 2 else nc.sync),
                    snap_out=snap_out,
                )

            # --- phase 2: D-step junction fixup from segment endpoints.
            v_aps = [vt[:, 0:BPC] for vt in v_tiles]
            for q in range(D - 1):
                new = full_step(v_aps, e2f[q], BPC, e_eng=nc.scalar)
                v_aps = [nv[:, 0:BPC] for nv in new]
            # last fixup step: tag block 0 only.
            etl = epool.tile([128, BPC], BF16, tag="el")
            nc.scalar.dma_start(etl[:], e2l[:, :])
            psl = pspool.tile([128, NCOLS], F32, tag="ps0")
            for kb in range(NB):
                nc.tensor.matmul(
                    psl[:, 0:BPC], mt_sb[:, kb * N_TAGS:kb * N_TAGS + 128],
                    v_aps[kb], start=(kb == 0), stop=(kb == NB - 1),
                )
            nvz = vpool.tile([128, BPC], BF16, tag="vz")
            nc.vector.tensor_mul(nvz[:], psl[:, 0:BPC], etl[:])
            nc.scalar.dma_start(zout[:, :], nvz[:])

    nc.compile()
    return nc


def _prepare_inputs(decoded, transitions):
    """Per-core input dicts + host-side sn (for D=1)."""
    decoded = np.asarray(decoded, dtype=np.float32)
    transitions = np.asarray(transitions, dtype=np.float32)

    M64 = np.exp(transitions.astype(np.float64))          # [next, prev]
    Mt_bf = np.ascontiguousarray(M64.T.astype(BF16_NP))   # [prev, next]
    E32 = np.exp(decoded - np.float32(CSCALE))            # fp32 [T, N]
    E = E32.astype(BF16_NP)
    w0 = M64.sum(axis=1) / N_TAGS                         # [N] fp64
    mstart = M64[:, START_IDX]                            # [N] fp64

    in_maps = []
    sn_host = []
    steps1 = np.arange(1, L)
    for c in range(NCORES):
        segs1 = np.minimum(c * BPC + np.arange(NCOLS), S - 1)
        segs2 = np.minimum(c * BPC + 1 + np.arange(BPC), S - 1)
        t1 = segs1 * L
        t2 = segs2 * L
        # state after step 0 (host): (M @ guess) .* e_0
        v1 = w0[:, None] * E32[t1].T.astype(np.float64)   # [N, NCOLS]
        if c == 0:
            v1[:, 0] = mstart * E32[0].astype(np.float64)
        v1 = v1.astype(BF16_NP)
        a1 = E[t1[None, :] + steps1[:, None]]             # [L-1, NCOLS, N]
        e1 = np.ascontiguousarray(
            a1.reshape(L - 1, NCOLS, NB, 128).transpose(0, 3, 2, 1)
        ).reshape(L - 1, 128, NB * NCOLS)
        im = {"mt": Mt_bf, "v1": v1, "e1": e1}
        if D >= 2:
            a2 = E[t2[None, :] + np.arange(D - 1)[:, None]]  # [D-1, BPC, N]
            im["e2f"] = np.ascontiguousarray(
                a2.reshape(D - 1, BPC, NB, 128).transpose(0, 3, 2, 1)
            ).reshape(D - 1, 128, NB * BPC)
        a2l = E[t2 + (D - 1)][:, 0:128]                   # [BPC, 128]
        im["e2l"] = np.ascontiguousarray(a2l.T)           # [128, BPC]
        in_maps.append(im)
        sn_host.append(v1[0:128, 1:BPC + 1].astype(np.float64))
    return in_maps, sn_host


def _assemble(transitions, results, sn_host):
    """Host-side kappa extraction + terminal logsumexp (tiny, fp64)."""
    kappa_sum = 0.0
    max_spread = 0.0
    for c in range(NCORES):
        z = results[c]["zout"].astype(np.float64)         # [128, BPC]
        if D >= 2:
            sn = results[c]["snap"].astype(np.float64)[:, 1:]  # [128, NCOLS-1]
        else:
            sn = sn_host[c]                               # [128, BPC]
        nj = BPC if c < NCORES - 1 else BPC - 1
        zv = z[:, :nj]
        sv = sn[:, :nj]
        valid = (zv > 0) & (sv > 0)
        with np.errstate(divide="ignore", invalid="ignore"):
            dlt = np.where(valid, np.log(zv) - np.log(sv), np.nan)
        kap = np.nanmedian(dlt, axis=0)
        spread = np.nanmax(dlt, axis=0) - np.nanmin(dlt, axis=0)
        max_spread = max(max_spread, float(np.nanmax(spread)))
        kappa_sum += float(kap.sum())

    y_last = results[NCORES - 1]["yend"].astype(np.float64)
    y_last = y_last.reshape(N_TAGS, NCOLS)[:, BPC - 1]
    with np.errstate(divide="ignore"):
        logx = np.log(y_last) + kappa_sum + CSCALE * SEQ_LEN
    term = logx + transitions[STOP_IDX].astype(np.float64)
    term = term[np.isfinite(term)]
    mx = term.max()
    alpha = mx + np.log(np.exp(term - mx).sum())
    return alpha, max_spread


def kernel(decoded, transitions, raw_outputs=None, outputs=None, _backend="hw"):
    transitions = np.asarray(transitions, dtype=np.float32)
    in_maps, sn_host = _prepare_inputs(decoded, transitions)
    _CACHE["in_maps"] = in_maps
    _CACHE["sn_host"] = sn_host

    if "nc" not in _CACHE:
        _CACHE["nc"] = _build_program()
    nc = _CACHE["nc"]

    if _backend == "sim":
        from concourse.bass_interp import CoreSim
        out_names = ["snap", "yend", "zout"] if D >= 2 else ["yend", "zout"]
        results = []
        for c in range(NCORES):
            sim = CoreSim(nc, trace=False)
            for k, v in in_maps[c].items():
                sim.tensor(k)[:] = v
            sim.simulate()
            results.append({k: np.array(sim.tensor(k)) for k in out_names})
    else:
        from concourse.bass_utils import run_bass_kernel_spmd
        res = run_bass_kernel_spmd(nc, in_maps, list(range(NCORES)))
        results = res.results

    alpha, max_spread = _assemble(transitions, results, sn_host)
    if max_spread > 0.5:
        import sys
        print(f"kernel: WARNING junction spread {max_spread:.3e}", file=sys.stderr)
    return np.float32(alpha)


# revision 6
# speedup vs baseline: 1.4423x; 1.0361x over previous
"""CRF forward-algorithm (log partition) kernel for 8 Trainium2 NeuronCores.

Strategy: segment-spliced exp-space scan (v3).

The reference recurrence  fv' = logsumexp_prev(fv + T) + feat  is, in exp
space, a linear matvec chain  v' = (M @ v) .* e_t  with M = exp(T) fixed.
We split the T=16384 steps into S=1024 segments of L=16 and run all segments
in parallel from a guess vector, batched 129 columns per core so the PE array
runs dense [128x128] x [128x129] matmuls.  Products of positive matrices
contract toward rank-1 (contraction factor ~0.04/step here), so the true
correction at each segment junction is a pure scalar kappa, measured by
re-running only the first D steps of each segment from the previous
segment's endpoint.  alpha = lse(final) + sum(kappa).

v3 refinements:
  - Step 0 of every segment is computed on the host (uniform guess ->
    state = rowsum(M)/N .* e_0, elementwise).  Device runs steps 1..15.
  - Fixup depth D=1 computing only tag-block 0 (the kappa median needs
    128 tags, not 1024); its reference state sn is the uploaded v1, so
    no snapshot output is needed.
  - The per-step state lives in ONE [128, 8*129] SBUF tile (slices per
    tag block), so the initial v1 load and the final yend store are
    single contiguous DMAs (2KB/partition rows, not 258B fragments).
  - PE warm-up matmuls run during the initial 2MB weight DMA to open
    the HAM clock-gate; the first device step is kb-major so matmuls
    chase the arriving weight sections, with the 8 per-block vector
    multiplies interleaved into the last kb pass.
  - PSUM tiles hold two step-parity regions per bank to decouple the
    vector engine's psum reads from next-step matmul writes.

Per-step rescaling is folded into the emissions as a constant e^-8; all
bookkeeping scales are recovered analytically at the end.  Each core is
fully independent (no collectives): core c owns segments [c*128, c*128+128]
(129 columns, one redundant boundary column so junction sources are always
core-local).  The host does the tiny O(S*N) final assembly in fp64.
"""

import numpy as np
import ml_dtypes

import concourse.bass as bass
import concourse.bacc as bacc
import concourse.mybir as mybir
import concourse.tile as tile

BF16_NP = ml_dtypes.bfloat16
BF16 = mybir.dt.bfloat16
F32 = mybir.dt.float32

SEQ_LEN = 16384
N_TAGS = 1024
START_IDX = 1022
STOP_IDX = 1023
NB = 8                 # 1024 tags = 8 blocks of 128 partitions
L = 16                 # segment length (steps)
D = 1                  # junction fixup depth (steps, >= 1)
S = SEQ_LEN // L       # 1024 segments
NCORES = 8
BPC = S // NCORES      # 128 segments owned per core
NCOLS = BPC + 1        # 129 phase-1 columns (1 redundant boundary col)
CSCALE = 8.0           # constant per-step rescale folded into emissions
NWARM = 48             # PE warm-up matmuls issued during the initial DMA

_CACHE = {}


def _build_program():
    nc = bacc.Bacc("TRN2", target_bir_lowering=False, debug=False)
    mt = nc.dram_tensor("mt", [N_TAGS, N_TAGS], BF16, kind="ExternalInput")
    v1 = nc.dram_tensor("v1", [128, NB * NCOLS], BF16, kind="ExternalInput")
    e1 = nc.dram_tensor("e1", [L - 1, 128, NB * NCOLS], BF16, kind="ExternalInput")
    if D >= 2:
        e2f = nc.dram_tensor("e2f", [D - 1, 128, NB * BPC], BF16, kind="ExternalInput")
        snap = nc.dram_tensor("snap", [128, NCOLS], BF16, kind="ExternalOutput")
    e2l = nc.dram_tensor("e2l", [128, BPC], BF16, kind="ExternalInput")
    yend = nc.dram_tensor("yend", [128, NB * NCOLS], BF16, kind="ExternalOutput")
    zout = nc.dram_tensor("zout", [128, BPC], BF16, kind="ExternalOutput")

    with tile.TileContext(nc) as tc:
        with (
            tc.tile_pool(name="mpool", bufs=1) as mpool,
            tc.tile_pool(name="vpool", bufs=2) as vpool,
            tc.tile_pool(name="epool", bufs=3) as epool,
            tc.tile_pool(name="pspool", bufs=1, space="PSUM") as pspool,
        ):
            def ps_tile(mb):
                # two step-parity regions per psum bank
                return pspool.tile([128, 2 * NCOLS], F32, tag=f"ps{mb}",
                                   name=f"ps{mb}")

            # --- PE warm-up: keep the HAM clock-gate open during the load.
            warm = mpool.tile([128, 128], BF16, tag="warm")
            nc.vector.memset(warm[:], 0.0)
            wps = ps_tile(0)
            for _ in range(NWARM):
                nc.tensor.matmul(wps[:, 0:128], warm[:], warm[:],
                                 start=True, stop=True)

            # --- input DMAs.  mt section 0 + the full v1 state first so the
            # kb-major first step can start as soon as they land.
            mt_sb = mpool.tile([128, NB * N_TAGS], BF16)
            vall = vpool.tile([128, NB * NCOLS], BF16, tag="vall")
            nc.sync.dma_start(mt_sb[:, 0:N_TAGS], mt[0:128, :])
            nc.scalar.dma_start(vall[:], v1[:, :])
            for kb in range(1, NB):
                nc.sync.dma_start(
                    mt_sb[:, kb * N_TAGS:(kb + 1) * N_TAGS],
                    mt[kb * 128:(kb + 1) * 128, :],
                )

            def vsl(vt, kb, ncols=NCOLS):
                return vt[:, kb * NCOLS:kb * NCOLS + ncols]

            # --- first device step (global step 1), kb-major with the
            # vector multiplies interleaved into the last kb pass.
            et0 = epool.tile([128, NB * NCOLS], BF16, tag="e")
            nc.scalar.dma_start(et0[:], e1[0])
            ps_list = [ps_tile(mb) for mb in range(NB)]
            vnew = vpool.tile([128, NB * NCOLS], BF16, tag="vall")
            for kb in range(NB):
                for mb in range(NB):
                    sec = kb * N_TAGS + mb * 128
                    nc.tensor.matmul(
                        ps_list[mb][:, 0:NCOLS], mt_sb[:, sec:sec + 128],
                        vsl(vall, kb),
                        start=(kb == 0), stop=(kb == NB - 1),
                    )
                    if kb == NB - 1:
                        nc.vector.tensor_mul(
                            vsl(vnew, mb), ps_list[mb][:, 0:NCOLS],
                            et0[:, mb * NCOLS:(mb + 1) * NCOLS])
            if D >= 2:
                nc.scalar.dma_start(snap[:, :], vsl(vnew, 0))
            vall = vnew

            def full_step(vold, e_row, ncols, parity, yend_out=False,
                          e_eng=None, snap_out=None):
                et = epool.tile([128, NB * ncols], BF16, tag="e")
                (e_eng or nc.sync).dma_start(et[:], e_row)
                vnew = vpool.tile([128, NB * NCOLS], BF16, tag="vall")
                po = parity * NCOLS
                for mb in range(NB):
                    ps = ps_tile(mb)
                    for kb in range(NB):
                        sec = kb * N_TAGS + mb * 128
                        nc.tensor.matmul(
                            ps[:, po:po + ncols], mt_sb[:, sec:sec + 128],
                            vsl(vold, kb, ncols),
                            start=(kb == 0), stop=(kb == NB - 1),
                        )
                    nc.vector.tensor_mul(
                        vsl(vnew, mb, ncols), ps[:, po:po + ncols],
                        et[:, mb * ncols:(mb + 1) * ncols])
                    if snap_out is not None and mb == 0:
                        nc.scalar.dma_start(snap_out, vsl(vnew, 0, ncols))
                if yend_out:
                    nc.scalar.dma_start(yend[:, :], vnew[:])
                return vnew

            # --- phase-1 device steps 2..15 (e1 rows 1..14).
            for r in range(1, L - 1):
                snap_out = snap[:, :] if (D >= 2 and r == D - 2) else None
                vall = full_step(
                    vall, e1[r], NCOLS, parity=r % 2,
                    yend_out=(r == L - 2),
                    e_eng=(nc.scalar if r % 2 else nc.sync),
                    snap_out=snap_out,
                )

            # --- phase 2: D-step junction fixup from segment endpoints.
            for q in range(D - 1):
                vall = full_step(vall, e2f[q], BPC, parity=(L + q) % 2,
                                 e_eng=nc.scalar)
            # last fixup step: tag block 0 only.
            etl = epool.tile([128, BPC], BF16, tag="el")
            nc.scalar.dma_start(etl[:], e2l[:, :])
            psl = ps_tile(0)
            po = ((L + D - 1) % 2) * NCOLS
            for kb in range(NB):
                nc.tensor.matmul(
                    psl[:, po:po + BPC], mt_sb[:, kb * N_TAGS:kb * N_TAGS + 128],
                    vsl(vall, kb, BPC), start=(kb == 0), stop=(kb == NB - 1),
                )
            nvz = vpool.tile([128, BPC], BF16, tag="vz")
            nc.vector.tensor_mul(nvz[:], psl[:, po:po + BPC], etl[:])
            nc.scalar.dma_start(zout[:, :], nvz[:])

    nc.compile()
    return nc


def _prepare_inputs(decoded, transitions):
    """Per-core input dicts + host-side sn (for D=1)."""
    decoded = np.asarray(decoded, dtype=np.float32)
    transitions = np.asarray(transitions, dtype=np.float32)

    M64 = np.exp(transitions.astype(np.float64))          # [next, prev]
    Mt_bf = np.ascontiguousarray(M64.T.astype(BF16_NP))   # [prev, next]
    E32 = np.exp(decoded - np.float32(CSCALE))            # fp32 [T, N]
    E = E32.astype(BF16_NP)
    w0 = M64.sum(axis=1) / N_TAGS                         # [N] fp64
    mstart = M64[:, START_IDX]                            # [N] fp64

    in_maps = []
    sn_host = []
    steps1 = np.arange(1, L)
    for c in range(NCORES):
        segs1 = np.minimum(c * BPC + np.arange(NCOLS), S - 1)
        segs2 = np.minimum(c * BPC + 1 + np.arange(BPC), S - 1)
        t1 = segs1 * L
        t2 = segs2 * L
        # state after step 0 (host): (M @ guess) .* e_0
        v1 = w0[:, None] * E32[t1].T.astype(np.float64)   # [N, NCOLS]
        if c == 0:
            v1[:, 0] = mstart * E32[0].astype(np.float64)
        v1 = v1.astype(BF16_NP)
        # device layout [part, kb*NCOLS + col]
        v1_dev = np.ascontiguousarray(
            v1.reshape(NB, 128, NCOLS).transpose(1, 0, 2)
        ).reshape(128, NB * NCOLS)
        a1 = E[t1[None, :] + steps1[:, None]]             # [L-1, NCOLS, N]
        e1 = np.ascontiguousarray(
            a1.reshape(L - 1, NCOLS, NB, 128).transpose(0, 3, 2, 1)
        ).reshape(L - 1, 128, NB * NCOLS)
        im = {"mt": Mt_bf, "v1": v1_dev, "e1": e1}
        if D >= 2:
            a2 = E[t2[None, :] + np.arange(D - 1)[:, None]]  # [D-1, BPC, N]
            im["e2f"] = np.ascontiguousarray(
                a2.reshape(D - 1, BPC, NB, 128).transpose(0, 3, 2, 1)
            ).reshape(D - 1, 128, NB * BPC)
        a2l = E[t2 + (D - 1)][:, 0:128]                   # [BPC, 128]
        im["e2l"] = np.ascontiguousarray(a2l.T)           # [128, BPC]
        in_maps.append(im)
        sn_host.append(v1[0:128, 1:BPC + 1].astype(np.float64))
    return in_maps, sn_host


def _assemble(transitions, results, sn_host):
    """Host-side kappa extraction + terminal logsumexp (tiny, fp64)."""
    kappa_sum = 0.0
    max_spread = 0.0
    for c in range(NCORES):
        z = results[c]["zout"].astype(np.float64)         # [128, BPC]
        if D >= 2:
            sn = results[c]["snap"].astype(np.float64)[:, 1:]  # [128, NCOLS-1]
        else:
            sn = sn_host[c]                               # [128, BPC]
        nj = BPC if c < NCORES - 1 else BPC - 1
        zv = z[:, :nj]
        sv = sn[:, :nj]
        valid = (zv > 0) & (sv > 0)
        with np.errstate(divide="ignore", invalid="ignore"):
            dlt = np.where(valid, np.log(zv) - np.log(sv), np.nan)
        kap = np.nanmedian(dlt, axis=0)
        spread = np.nanmax(dlt, axis=0) - np.nanmin(dlt, axis=0)
        max_spread = max(max_spread, float(np.nanmax(spread)))
        kappa_sum += float(kap.sum())

    # yend layout [part, kb*NCOLS + col] -> tag = kb*128 + part
    y = results[NCORES - 1]["yend"].astype(np.float64)
    y_last = y.reshape(128, NB, NCOLS).transpose(1, 0, 2).reshape(
        N_TAGS, NCOLS)[:, BPC - 1]
    with np.errstate(divide="ignore"):
        logx = np.log(y_last) + kappa_sum + CSCALE * SEQ_LEN
    term = logx + transitions[STOP_IDX].astype(np.float64)
    term = term[np.isfinite(term)]
    mx = term.max()
    alpha = mx + np.log(np.exp(term - mx).sum())
    return alpha, max_spread


def kernel(decoded, transitions, raw_outputs=None, outputs=None, _backend="hw"):
    transitions = np.asarray(transitions, dtype=np.float32)
    in_maps, sn_host = _prepare_inputs(decoded, transitions)
    _CACHE["in_maps"] = in_maps
    _CACHE["sn_host"] = sn_host

    if "nc" not in _CACHE:
        _CACHE["nc"] = _build_program()
    nc = _CACHE["nc"]

    if _backend == "sim":
        from concourse.bass_interp import CoreSim
        out_names = ["snap", "yend", "zout"] if D >= 2 else ["yend", "zout"]
        results = []
        for c in range(NCORES):
            sim = CoreSim(nc, trace=False)
            for k, v in in_maps[c].items():
                sim.tensor(k)[:] = v
            sim.simulate()
            results.append({k: np.array(sim.tensor(k)) for k in out_names})
    else:
        from concourse.bass_utils import run_bass_kernel_spmd
        res = run_bass_kernel_spmd(nc, in_maps, list(range(NCORES)))
        results = res.results

    alpha, max_spread = _assemble(transitions, results, sn_host)
    if max_spread > 1.0:
        import sys
        print(f"kernel: WARNING junction spread {max_spread:.3e}", file=sys.stderr)
    return np.float32(alpha)
